# revision 31
# baseline (speedup 1.0000x reference)
"""Trainium2 Bass kernel for a full attention block (QKV proj + RMSNorm + RoPE +
softmax attention + output proj), batch-data-parallel across 8 NeuronCores.

Shapes (hardcoded): x (8, 1024, 1024), H=16 heads, hd=64.
Each core processes one batch element; weights are replicated.

Head-group (8-head) pipelined schedule so ACT exp overlaps PE matmuls of the
other half; all matmul operands bf16 (psum stays f32); qkv psum staged to a
bf16 SBUF tile with one copy (fast bank recycle + DVE 2x-mode rope reads);
RoPE uses a swapped-halves AP view (one multiply for the rotated term);
q-RMS rsqrt batched per slice (Sqrt stays out of the exp table-set's way)
and folded in-place into q; k-RMS folded into the exp scale (per-partition)
as 1/sqrt(ss+hd*eps), absorbing the 1/sqrt(hd) softmax scale; S matmuls use
persistent zero-padded K stationaries; PV uses a 128-padded V with a ones
column so softmax sums land in psum row 64 and normalization is a
reciprocal + gpsimd partition broadcast; transposes (f32r) evacuated in
batches of 4 via DVE (ACT must not read f32r).
"""
import numpy as np
import ml_dtypes

import concourse.bass as bass
from concourse import bacc
import concourse.mybir as mybir
import concourse.tile as tile
from concourse.bass_utils import run_bass_kernel_spmd
from concourse.masks import make_identity

F32 = mybir.dt.float32
F32R = mybir.dt.float32r
BF16 = mybir.dt.bfloat16
AF = mybir.ActivationFunctionType
ALU = mybir.AluOpType

B, L, C, H, HD = 8, 1024, 1024, 16, 64
EPS = 1e-6
NLB = L // 128   # 8 l-blocks
NCB = C // 128   # 8 c-blocks
NJB = L // 128   # 8 j-blocks
N_CORES = 8

_nc_cache = None
_last_results = None  # BassKernelResults of the most recent run (for test.py)


def _bcast(ap2d, reps):
    """(128, w) AP -> (128, reps, w) stride-0 broadcast view."""
    return bass.AP(tensor=ap2d.tensor, offset=ap2d.offset,
                   ap=[ap2d.ap[0], [0, reps], ap2d.ap[1]])


def _bcast_rot(ap2d, reps):
    """(128, 64) AP -> (128, reps, 2, 32) stride-0 bcast view (hd split)."""
    return bass.AP(tensor=ap2d.tensor, offset=ap2d.offset,
                   ap=[ap2d.ap[0], [0, reps], [32, 2], [1, 32]])


def _rot_view(ps):
    """(128, 512) psum AP -> (128, 8, 2, 32) view with 32-halves swapped
    within each 64-wide head: element (p,h,r,j) = ps[p, 64h + 32(1-r) + j]."""
    return bass.AP(tensor=ps.tensor, offset=ps.offset + 32,
                   ap=[ps.ap[0], [HD, 8], [-32, 2], [1, 32]])


def _inner_bcast(ap2d, reps):
    """(128, w) AP -> (128, w, reps) stride-0 inner broadcast view."""
    return bass.AP(tensor=ap2d.tensor, offset=ap2d.offset,
                   ap=[ap2d.ap[0], ap2d.ap[1], [0, reps]])


def build_nc():
    nc = bacc.Bacc("TRN2", target_bir_lowering=False)

    xT = nc.declare_dram_parameter("xT", [C, L], BF16, isOutput=False)
    wq = nc.declare_dram_parameter("wq", [C, 3 * C], BF16, isOutput=False)
    wp = nc.declare_dram_parameter("wp", [C, C], BF16, isOutput=False)
    # RoPE tables with rms-norm weights folded in (host-prepared)
    cq = nc.declare_dram_parameter("cq", [L, HD], BF16, isOutput=False)
    sq = nc.declare_dram_parameter("sq", [L, HD], BF16, isOutput=False)
    ck = nc.declare_dram_parameter("ck", [L, HD], BF16, isOutput=False)
    sk = nc.declare_dram_parameter("sk", [L, HD], BF16, isOutput=False)
    y = nc.declare_dram_parameter("y", [L, C], F32, isOutput=True)

    def tab_view(t):
        # (L, 64) DRAM -> SBUF (128, 8, 64): element (p, lc, j) = t[128*lc + p, j]
        return bass.AP(tensor=t, offset=0,
                       ap=[[HD, 128], [128 * HD, NLB], [1, HD]])

    with tile.TileContext(nc) as tc:
        with tc.tile_pool(name="persist", bufs=1) as persist:
            # --- persistent tiles ---
            cq_sb = persist.tile([128, NLB, HD], BF16)
            sq_sb = persist.tile([128, NLB, HD], BF16)
            ck_sb = persist.tile([128, NLB, HD], BF16)
            sk_sb = persist.tile([128, NLB, HD], BF16)
            nc.sync.dma_start(out=cq_sb, in_=tab_view(cq))
            nc.sync.dma_start(out=sq_sb, in_=tab_view(sq))
            nc.sync.dma_start(out=ck_sb, in_=tab_view(ck))
            nc.sync.dma_start(out=sk_sb, in_=tab_view(sk))

            # V blocks padded to 128 cols (65th = ones, rest zero) so PV
            # matmuls write a full 128-partition psum
            vb = persist.tile([128, NLB, H, 128], BF16)
            nc.vector.memset(vb, 0.0)
            ident_f = persist.tile([128, 128], F32)
            make_identity(nc, ident_f)
            ident = persist.tile([128, 128], F32R)
            nc.vector.tensor_copy(ident, ident_f)
            # zero-padded K stationaries: partitions [64i, 64i+64) hold the
            # current head's K^T, the other half stays zero forever
            kz0 = persist.tile([128, L], BF16)
            nc.vector.memset(kz0, 0.0)
            kz1 = persist.tile([128, L], BF16)
            nc.vector.memset(kz1, 0.0)
            kzs = (kz0, kz1)
            eps_q = persist.tile([128, 1], F32)
            nc.vector.memset(eps_q, EPS)
            eps_k = persist.tile([128, 1], F32)
            nc.vector.memset(eps_k, HD * EPS)

            ones128 = persist.tile([128, 1], F32)
            nc.vector.memset(ones128, 1.0)
            nc.vector.tensor_copy(
                bass.AP(tensor=vb.tensor, offset=vb.offset + HD,
                        ap=[vb.ap[0], vb.ap[1], vb.ap[2], [1, 1]]),
                bass.AP(tensor=ones128.tensor, offset=ones128.offset,
                        ap=[ones128.ap[0], [0, NLB], [0, H], [1, 1]]))

            with tc.tile_pool(name="pbig", bufs=1) as pbig, \
                 tc.tile_pool(name="phalf", bufs=2) as phalf, \
                 tc.tile_pool(name="pw", bufs=2) as pw, \
                 tc.tile_pool(name="ppt", bufs=3) as ppt, \
                 tc.tile_pool(name="p1", bufs=1) as p1, \
                 tc.tile_pool(name="ps_a", bufs=2, space="PSUM") as ps_a, \
                 tc.tile_pool(name="ps_s", bufs=1, space="PSUM") as ps_s, \
                 tc.tile_pool(name="ps_o", bufs=1, space="PSUM") as ps_o:

                xr = pbig.tile([128, NCB, L], BF16, name="xr")
                outT = pbig.tile([128, NCB, L], BF16, name="outT")
                # element (p, cc, l) = xT[128*cc + p, l]; split in half so the
                # first matmuls can start as soon as cb 0-3 land
                for xh in range(2):
                    nc.sync.dma_start(out=xr[:, 4 * xh:4 * (xh + 1), :],
                                      in_=bass.AP(
                        tensor=xT, offset=512 * L * xh,
                        ap=[[L, 128], [128 * L, 4], [1, L]]))

                for hg in range(2):
                    with nc.named_scope(f"qkv{hg}"):
                        qr = phalf.tile([128, NLB, 512], F32R, tag="qr",
                                        bufs=1, name=f"qr{hg}")
                        kr = phalf.tile([128, NLB, 512], F32R, tag="kr",
                                        bufs=1, name=f"kr{hg}")
                        for kind, n in (("q", hg), ("k", hg + 2), ("v", hg + 4)):
                            wqn = pw.tile([128, NCB, 512], BF16, tag="wqn",
                                          name=f"w_{kind}{hg}")
                            # element (p, cc, j) = wq[128*cc + p, 512*n + j]
                            nc.sync.dma_start(out=wqn, in_=bass.AP(
                                tensor=wq, offset=512 * n,
                                ap=[[3 * C, 128], [128 * 3 * C, NCB], [1, 512]]))
                            if kind != "v":
                                sst_all = p1.tile([128, NLB, 8], F32,
                                                  tag="sst", bufs=2)
                            for lb in range(NLB):
                                ps = ps_a.tile([128, 512], F32, tag="psa")
                                for cb in range(NCB):
                                    nc.tensor.matmul(
                                        ps,
                                        lhsT=xr[:, cb, 128 * lb:128 * (lb + 1)],
                                        rhs=wqn[:, cb, :],
                                        start=(cb == 0), stop=(cb == NCB - 1))
                                if kind == "v":
                                    nc.any.tensor_copy(
                                        vb[:, lb, 8 * hg:8 * hg + 8, 0:HD],
                                        ps.rearrange("p (h d) -> p h d", d=HD))
                                    continue
                                # stage psum out fast (frees the bank after
                                # one copy instead of three DVE reads)
                                stg = p1.tile([128, 512], BF16, tag="stage",
                                              bufs=3)
                                nc.any.tensor_copy(stg, ps)
                                # rms stats (Square shares the exp ACT table
                                # set; Sqrt is deferred + batched per slice)
                                sqt = p1.tile([128, 512], BF16, tag="sqt",
                                              bufs=2)
                                nc.scalar.activation(sqt, stg, AF.Square)
                                nc.vector.tensor_reduce(
                                    sst_all[:, lb, :],
                                    sqt.rearrange("p (h d) -> p h d", d=HD),
                                    axis=mybir.AxisListType.X, op=ALU.add)
                                s3 = stg.rearrange("p (h d) -> p h d", d=HD)
                                cos_sb = cq_sb if kind == "q" else ck_sb
                                sin_sb = sq_sb if kind == "q" else sk_sb
                                dst = qr if kind == "q" else kr
                                a_t = p1.tile([128, 8, HD], BF16,
                                              tag="ropeA", bufs=2)
                                nc.vector.tensor_mul(
                                    a_t, s3, _bcast(cos_sb[:, lb, :], 8))
                                b_t = p1.tile([128, 8, 2, 32], BF16,
                                              tag="ropeB", bufs=2)
                                nc.vector.tensor_mul(
                                    b_t, _rot_view(stg),
                                    _bcast_rot(sin_sb[:, lb, :], 8))
                                nc.vector.tensor_add(
                                    dst[:, lb, :].rearrange(
                                        "p (h d) -> p h d", d=HD),
                                    a_t,
                                    b_t.rearrange("p h r j -> p h (r j)"))
                            # batched slice-wide rsqrt (one Sqrt per slice
                            # keeps ACT table switches rare)
                            if kind == "q":
                                rt = p1.tile([128, NLB, 8], F32, tag="rt",
                                             bufs=2)
                                nc.scalar.activation(
                                    rt.rearrange("p l h -> p (l h)"),
                                    sst_all.rearrange("p l h -> p (l h)"),
                                    AF.Sqrt, scale=1.0 / HD, bias=eps_q)
                                fqv = p1.tile([128, NLB, 8], F32, tag="fqv",
                                              bufs=2)
                                nc.vector.reciprocal_approx_fast(
                                    fqv.rearrange("p l h -> p (l h)"),
                                    rt.rearrange("p l h -> p (l h)"))
                                for lb in range(NLB):
                                    # fold fq into q in place
                                    nc.vector.tensor_mul(
                                        qr[:, lb, :].rearrange(
                                            "p (h d) -> p h d", d=HD),
                                        qr[:, lb, :].rearrange(
                                            "p (h d) -> p h d", d=HD),
                                        _inner_bcast(fqv[:, lb, :], HD))
                            elif kind == "k":
                                # fk = 1/sqrt(ss + hd*eps)  (= rms_k/8;
                                # absorbs the 1/sqrt(hd) softmax scale),
                                # folded into k in place so the softmax exp
                                # needs no per-partition scale
                                rt = p1.tile([128, NLB, 8], F32, tag="rt",
                                             bufs=2)
                                nc.scalar.activation(
                                    rt.rearrange("p l h -> p (l h)"),
                                    sst_all.rearrange("p l h -> p (l h)"),
                                    AF.Sqrt, scale=1.0, bias=eps_k)
                                fkv = p1.tile([128, NLB, 8], F32, tag="fqv",
                                              bufs=2)
                                nc.vector.reciprocal_approx_fast(
                                    fkv.rearrange("p l h -> p (l h)"),
                                    rt.rearrange("p l h -> p (l h)"))
                                for lb in range(NLB):
                                    nc.vector.tensor_mul(
                                        kr[:, lb, :].rearrange(
                                            "p (h d) -> p h d", d=HD),
                                        kr[:, lb, :].rearrange(
                                            "p (h d) -> p h d", d=HD),
                                        _inner_bcast(fkv[:, lb, :], HD))

                    # ---- transposes for this head group ----
                    qT = phalf.tile([128, 4, L], BF16, tag="qT", name=f"qT{hg}")
                    kT = phalf.tile([128, 4, L], BF16, tag="kT", name=f"kT{hg}")
                    with nc.named_scope(f"tr{hg}"):
                        for src, dstT in ((qr, qT), (kr, kT)):
                            for dc in range(4):
                                for lq in range(2):
                                    pt_ps = ps_a.tile([128, 512], F32R,
                                                      tag="psa")
                                    for t in range(4):
                                        lb = 4 * lq + t
                                        nc.tensor.transpose(
                                            pt_ps[:, 128 * t:128 * (t + 1)],
                                            src[:, lb,
                                                128 * dc:128 * (dc + 1)],
                                            ident)
                                    # explicit DVE: ACT must not read f32r
                                    nc.vector.tensor_copy(
                                        dstT[:, dc,
                                             512 * lq:512 * (lq + 1)], pt_ps)

                    # ---- attention for the 8 heads of this group ----
                    with nc.named_scope(f"attn{hg}"):
                        for cbl in range(4):
                            cb = 4 * hg + cbl
                            for i in range(2):
                                h = 2 * cb + i
                                hp = 64 * i
                                nc.vector.tensor_copy(
                                    kzs[i][hp:hp + HD, :],
                                    kT[hp:hp + HD, cbl, :])
                                pso = ps_o.tile([128, L], F32, tag="pso",
                                                name=f"pso_{h}")
                                for jp in range(NJB // 2):
                                    # two j-blocks share one psum tile so a
                                    # single wide unscaled exp covers both
                                    pt_sb = ppt.tile([128, 2, L], BF16,
                                                     tag="pt")
                                    ss = ps_s.tile([128, 2, L], F32,
                                                   tag="pss")
                                    for d in range(2):
                                        jb = 2 * jp + d
                                        for hf in range(2):
                                            nc.tensor.matmul(
                                                ss[:, d,
                                                   512 * hf:512 * (hf + 1)],
                                                lhsT=kzs[i][
                                                    :, 128 * jb:128 * (jb + 1)],
                                                rhs=qT[:, cbl,
                                                       512 * hf:512 * (hf + 1)],
                                                start=True, stop=True)
                                    nc.scalar.activation(pt_sb, ss, AF.Exp)
                                    for d in range(2):
                                        jb = 2 * jp + d
                                        for hf in range(2):
                                            nc.tensor.matmul(
                                                pso[:,
                                                    512 * hf:512 * (hf + 1)],
                                                lhsT=vb[:, jb, h, :],
                                                rhs=pt_sb[:, d,
                                                          512 * hf:
                                                          512 * (hf + 1)],
                                                start=(jb == 0),
                                                stop=(jb == NJB - 1))
                                srow = p1.tile([1, L], F32, tag="srow",
                                               bufs=1)
                                nc.vector.tensor_copy(srow, pso[HD:HD + 1, :])
                                rs = p1.tile([1, L], F32, tag="rs", bufs=1)
                                nc.vector.reciprocal_approx_fast(rs, srow)
                                fsb = p1.tile([HD, L], F32, tag="fsb", bufs=2)
                                nc.gpsimd.partition_broadcast(fsb, rs)
                                nc.vector.tensor_mul(
                                    outT[hp:hp + HD, cb, :],
                                    pso[0:HD, :], fsb)

                # ---------------- output projection ----------------
                with nc.named_scope("proj"):
                    for hf in range(2):
                        wpn = pw.tile([128, NCB, 512], BF16, tag="wqn",
                                      name=f"wp{hf}")
                        nc.sync.dma_start(out=wpn, in_=bass.AP(
                            tensor=wp, offset=512 * hf,
                            ap=[[C, 128], [128 * C, NCB], [1, 512]]))
                        for lb in range(NLB):
                            psy = ps_a.tile([128, 512], F32, tag="psa")
                            for cb in range(NCB):
                                nc.tensor.matmul(
                                    psy,
                                    lhsT=outT[:, cb, 128 * lb:128 * (lb + 1)],
                                    rhs=wpn[:, cb, :],
                                    start=(cb == 0), stop=(cb == NCB - 1))
                            ysb = p1.tile([128, 512], F32, tag="ysb", bufs=2)
                            nc.any.tensor_copy(ysb, psy)
                            nc.sync.dma_start(
                                out=y[128 * lb:128 * (lb + 1),
                                      512 * hf:512 * (hf + 1)],
                                in_=ysb)

    nc.compile()
    return nc


def _get_nc():
    global _nc_cache
    if _nc_cache is None:
        _nc_cache = build_nc()
    return _nc_cache


def _host_prep(x, cos, sin, w_qkv, w_proj, q_norm_w, k_norm_w):
    x = np.asarray(x, dtype=np.float32)
    cos = np.asarray(cos, dtype=np.float32)
    sin = np.asarray(sin, dtype=np.float32)
    w_qkv = np.asarray(w_qkv, dtype=np.float32)
    w_proj = np.asarray(w_proj, dtype=np.float32)
    q_norm_w = np.asarray(q_norm_w, dtype=np.float32)
    k_norm_w = np.asarray(k_norm_w, dtype=np.float32)

    bf16 = ml_dtypes.bfloat16
    wqT = np.ascontiguousarray(w_qkv.T.astype(bf16))     # (C, 3C)
    wpT = np.ascontiguousarray(w_proj.T.astype(bf16))    # (C, C)

    def fold(w):
        # cosW[l,d] = cos[l,d]*w[d]
        # sinW[l,d<32] = -sin[l,d]*w[d+32]; sinW[l,d>=32] = sin[l,d]*w[d-32]
        cosW = cos * w[None, :]
        w_rot = np.concatenate([w[32:], w[:32]])
        sinW = (sin * w_rot[None, :]).copy()
        sinW[:, :32] *= -1.0
        return (np.ascontiguousarray(cosW.astype(bf16)),
                np.ascontiguousarray(sinW.astype(bf16)))

    cqt, sqt = fold(q_norm_w)
    ckt, skt = fold(k_norm_w)

    in_maps = []
    for b in range(N_CORES):
        in_maps.append({
            "xT": np.ascontiguousarray(x[b].T.astype(bf16)),
            "wq": wqT, "wp": wpT,
            "cq": cqt, "sq": sqt, "ck": ckt, "sk": skt,
        })
    return in_maps


def kernel(x, cos, sin, w_qkv, w_proj, q_norm_w, k_norm_w, _trace=False):
    global _last_results
    nc = _get_nc()
    in_maps = _host_prep(x, cos, sin, w_qkv, w_proj, q_norm_w, k_norm_w)
    r = run_bass_kernel_spmd(nc, in_maps, list(range(N_CORES)), trace=_trace)
    _last_results = r
    return np.stack([r.results[b]["y"] for b in range(N_CORES)], axis=0)


# revision 32
# speedup vs baseline: 1.0119x; 1.0119x over previous
"""Trainium2 Bass kernel for a full attention block (QKV proj + RMSNorm + RoPE +
softmax attention + output proj), batch-data-parallel across 8 NeuronCores.

Shapes (hardcoded): x (8, 1024, 1024), H=16 heads, hd=64.
Each core processes one batch element; weights are replicated.

v2: head-group (8-head) pipelined schedule so ACT exp overlaps PE matmuls of
the other half; all matmul operands bf16 (psum stays f32); S matmuls use
K=64 tile-positioned stationaries straight out of kT (no zero-padding);
RoPE reads PSUM directly with a swapped-halves AP view (one multiply for the
rotated term); q-RMS factor folded in with a single stride-0 broadcast mul;
k-RMS folded into the exp scale (per-partition) as 1/sqrt(ss+hd*eps) which
also absorbs the 1/sqrt(hd) softmax scale; PV uses a ones-augmented V so
softmax sums land in psum row 64; transposes evacuated in batches of 4.
"""
import numpy as np
import ml_dtypes

import concourse.bass as bass
from concourse import bacc
import concourse.mybir as mybir
import concourse.tile as tile
from concourse.bass_utils import run_bass_kernel_spmd
from concourse.masks import make_identity

F32 = mybir.dt.float32
F32R = mybir.dt.float32r
BF16 = mybir.dt.bfloat16
AF = mybir.ActivationFunctionType
ALU = mybir.AluOpType

B, L, C, H, HD = 8, 1024, 1024, 16, 64
EPS = 1e-6
NLB = L // 128   # 8 l-blocks
NCB = C // 128   # 8 c-blocks
NJB = L // 128   # 8 j-blocks
N_CORES = 8

_nc_cache = None
_last_results = None  # BassKernelResults of the most recent run (for test.py)


def _bcast(ap2d, reps):
    """(128, w) AP -> (128, reps, w) stride-0 broadcast view."""
    return bass.AP(tensor=ap2d.tensor, offset=ap2d.offset,
                   ap=[ap2d.ap[0], [0, reps], ap2d.ap[1]])


def _bcast_rot(ap2d, reps):
    """(128, 64) AP -> (128, reps, 2, 32) stride-0 bcast view (hd split)."""
    return bass.AP(tensor=ap2d.tensor, offset=ap2d.offset,
                   ap=[ap2d.ap[0], [0, reps], [32, 2], [1, 32]])


def _rot_view(ps):
    """(128, 512) psum AP -> (128, 8, 2, 32) view with 32-halves swapped
    within each 64-wide head: element (p,h,r,j) = ps[p, 64h + 32(1-r) + j]."""
    return bass.AP(tensor=ps.tensor, offset=ps.offset + 32,
                   ap=[ps.ap[0], [HD, 8], [-32, 2], [1, 32]])


def _inner_bcast(ap2d, reps):
    """(128, w) AP -> (128, w, reps) stride-0 inner broadcast view."""
    return bass.AP(tensor=ap2d.tensor, offset=ap2d.offset,
                   ap=[ap2d.ap[0], ap2d.ap[1], [0, reps]])


def build_nc():
    nc = bacc.Bacc("TRN2", target_bir_lowering=False)

    xT = nc.declare_dram_parameter("xT", [C, L], BF16, isOutput=False)
    wq = nc.declare_dram_parameter("wq", [C, 3 * C], BF16, isOutput=False)
    wp = nc.declare_dram_parameter("wp", [C, C], BF16, isOutput=False)
    # RoPE tables with rms-norm weights folded in (host-prepared)
    cq = nc.declare_dram_parameter("cq", [L, HD], BF16, isOutput=False)
    sq = nc.declare_dram_parameter("sq", [L, HD], BF16, isOutput=False)
    ck = nc.declare_dram_parameter("ck", [L, HD], BF16, isOutput=False)
    sk = nc.declare_dram_parameter("sk", [L, HD], BF16, isOutput=False)
    y = nc.declare_dram_parameter("y", [L, C], F32, isOutput=True)

    def tab_view(t):
        # (L, 64) DRAM -> SBUF (128, 8, 64): element (p, lc, j) = t[128*lc + p, j]
        return bass.AP(tensor=t, offset=0,
                       ap=[[HD, 128], [128 * HD, NLB], [1, HD]])

    with tile.TileContext(nc) as tc:
        with tc.tile_pool(name="persist", bufs=1) as persist:
            # --- persistent tiles ---
            cq_sb = persist.tile([128, NLB, HD], BF16)
            sq_sb = persist.tile([128, NLB, HD], BF16)
            ck_sb = persist.tile([128, NLB, HD], BF16)
            sk_sb = persist.tile([128, NLB, HD], BF16)
            nc.sync.dma_start(out=cq_sb, in_=tab_view(cq))
            nc.sync.dma_start(out=sq_sb, in_=tab_view(sq))
            nc.sync.dma_start(out=ck_sb, in_=tab_view(ck))
            nc.sync.dma_start(out=sk_sb, in_=tab_view(sk))

            fk_all = persist.tile([128, NLB, H], F32)        # fk per (j, head)
            # V blocks padded to 128 cols (65th = ones, rest zero) so PV
            # matmuls write a full 128-partition psum
            vb = persist.tile([128, NLB, H, 128], BF16)
            nc.vector.memset(vb, 0.0)
            ident_f = persist.tile([128, 128], F32)
            make_identity(nc, ident_f)
            ident = persist.tile([128, 128], F32R)
            nc.vector.tensor_copy(ident, ident_f)
            # zero-padded K stationaries: partitions [64i, 64i+64) hold the
            # current head's K^T, the other half stays zero forever
            kz0 = persist.tile([128, L], BF16)
            nc.vector.memset(kz0, 0.0)
            kz1 = persist.tile([128, L], BF16)
            nc.vector.memset(kz1, 0.0)
            kzs = (kz0, kz1)
            eps_q = persist.tile([128, 1], F32)
            nc.vector.memset(eps_q, EPS)
            eps_k = persist.tile([128, 1], F32)
            nc.vector.memset(eps_k, HD * EPS)

            ones128 = persist.tile([128, 1], F32)
            nc.vector.memset(ones128, 1.0)
            nc.vector.tensor_copy(
                bass.AP(tensor=vb.tensor, offset=vb.offset + HD,
                        ap=[vb.ap[0], vb.ap[1], vb.ap[2], [1, 1]]),
                bass.AP(tensor=ones128.tensor, offset=ones128.offset,
                        ap=[ones128.ap[0], [0, NLB], [0, H], [1, 1]]))

            with tc.tile_pool(name="pbig", bufs=1) as pbig, \
                 tc.tile_pool(name="phalf", bufs=2) as phalf, \
                 tc.tile_pool(name="pw", bufs=2) as pw, \
                 tc.tile_pool(name="ppt", bufs=4) as ppt, \
                 tc.tile_pool(name="p1", bufs=1) as p1, \
                 tc.tile_pool(name="ps_a", bufs=2, space="PSUM") as ps_a, \
                 tc.tile_pool(name="ps_s", bufs=2, space="PSUM") as ps_s, \
                 tc.tile_pool(name="ps_o", bufs=1, space="PSUM") as ps_o:

                xr = pbig.tile([128, NCB, L], BF16, name="xr")
                outT = pbig.tile([128, NCB, L], BF16, name="outT")
                # element (p, cc, l) = xT[128*cc + p, l]; split in half so
                # the first matmuls start as soon as cb 0-3 land
                for xh in range(2):
                    nc.sync.dma_start(out=xr[:, 4 * xh:4 * (xh + 1), :],
                                      in_=bass.AP(
                        tensor=xT, offset=512 * L * xh,
                        ap=[[L, 128], [128 * L, 4], [1, L]]))

                for hg in range(2):
                    with nc.named_scope(f"qkv{hg}"):
                        qr = phalf.tile([128, NLB, 512], F32R, tag="qr",
                                        bufs=1, name=f"qr{hg}")
                        kr = phalf.tile([128, NLB, 512], F32R, tag="kr",
                                        bufs=1, name=f"kr{hg}")
                        for kind, n in (("q", hg), ("k", hg + 2), ("v", hg + 4)):
                            wqn = pw.tile([128, NCB, 512], BF16, tag="wqn",
                                          name=f"w_{kind}{hg}")
                            # element (p, cc, j) = wq[128*cc + p, 512*n + j]
                            nc.sync.dma_start(out=wqn, in_=bass.AP(
                                tensor=wq, offset=512 * n,
                                ap=[[3 * C, 128], [128 * 3 * C, NCB], [1, 512]]))
                            if kind != "v":
                                sst_all = p1.tile([128, NLB, 8], F32,
                                                  tag="sst", bufs=2)
                            for lb in range(NLB):
                                ps = ps_a.tile([128, 512], F32, tag="psa")
                                for cb in range(NCB):
                                    nc.tensor.matmul(
                                        ps,
                                        lhsT=xr[:, cb, 128 * lb:128 * (lb + 1)],
                                        rhs=wqn[:, cb, :],
                                        start=(cb == 0), stop=(cb == NCB - 1))
                                if kind == "v":
                                    nc.any.tensor_copy(
                                        vb[:, lb, 8 * hg:8 * hg + 8, 0:HD],
                                        ps.rearrange("p (h d) -> p h d", d=HD))
                                    continue
                                # stage psum out fast (frees the bank after
                                # one copy instead of three DVE reads)
                                stg = p1.tile([128, 512], BF16, tag="stage",
                                              bufs=3)
                                nc.any.tensor_copy(stg, ps)
                                # rms stats (Square shares the exp ACT table
                                # set; Sqrt is deferred + batched per slice)
                                sqt = p1.tile([128, 512], BF16, tag="sqt",
                                              bufs=2)
                                nc.scalar.activation(sqt, stg, AF.Square)
                                nc.vector.tensor_reduce(
                                    sst_all[:, lb, :],
                                    sqt.rearrange("p (h d) -> p h d", d=HD),
                                    axis=mybir.AxisListType.X, op=ALU.add)
                                s3 = stg.rearrange("p (h d) -> p h d", d=HD)
                                cos_sb = cq_sb if kind == "q" else ck_sb
                                sin_sb = sq_sb if kind == "q" else sk_sb
                                dst = qr if kind == "q" else kr
                                a_t = p1.tile([128, 8, HD], BF16,
                                              tag="ropeA", bufs=2)
                                nc.vector.tensor_mul(
                                    a_t, s3, _bcast(cos_sb[:, lb, :], 8))
                                b_t = p1.tile([128, 8, 2, 32], BF16,
                                              tag="ropeB", bufs=2)
                                nc.vector.tensor_mul(
                                    b_t, _rot_view(stg),
                                    _bcast_rot(sin_sb[:, lb, :], 8))
                                nc.vector.tensor_add(
                                    dst[:, lb, :].rearrange(
                                        "p (h d) -> p h d", d=HD),
                                    a_t,
                                    b_t.rearrange("p h r j -> p h (r j)"))
                            # batched slice-wide rsqrt (one Sqrt per slice
                            # keeps ACT table switches rare)
                            if kind == "q":
                                rt = p1.tile([128, NLB, 8], F32, tag="rt",
                                             bufs=2)
                                nc.scalar.activation(
                                    rt.rearrange("p l h -> p (l h)"),
                                    sst_all.rearrange("p l h -> p (l h)"),
                                    AF.Sqrt, scale=1.0 / HD, bias=eps_q)
                                fqv = p1.tile([128, NLB, 8], F32, tag="fqv",
                                              bufs=2)
                                nc.vector.reciprocal_approx_fast(
                                    fqv.rearrange("p l h -> p (l h)"),
                                    rt.rearrange("p l h -> p (l h)"))
                                for lb in range(NLB):
                                    # fold fq into q in place
                                    nc.vector.tensor_mul(
                                        qr[:, lb, :].rearrange(
                                            "p (h d) -> p h d", d=HD),
                                        qr[:, lb, :].rearrange(
                                            "p (h d) -> p h d", d=HD),
                                        _inner_bcast(fqv[:, lb, :], HD))
                            elif kind == "k":
                                # fk = 1/sqrt(ss + hd*eps)  (= rms_k/8;
                                # absorbs the 1/sqrt(hd) softmax scale)
                                rt = p1.tile([128, NLB, 8], F32, tag="rt",
                                             bufs=2)
                                nc.scalar.activation(
                                    rt.rearrange("p l h -> p (l h)"),
                                    sst_all.rearrange("p l h -> p (l h)"),
                                    AF.Sqrt, scale=1.0, bias=eps_k)
                                nc.vector.reciprocal_approx_fast(
                                    fk_all[:, :, 8 * hg:8 * hg + 8],
                                    rt)

                    # ---- transposes for this head group ----
                    qT = phalf.tile([128, 4, L], BF16, tag="qT", name=f"qT{hg}")
                    kT = phalf.tile([128, 4, L], BF16, tag="kT", name=f"kT{hg}")
                    with nc.named_scope(f"tr{hg}"):
                        for src, dstT in ((qr, qT), (kr, kT)):
                            for dc in range(4):
                                for lq in range(2):
                                    pt_ps = ps_a.tile([128, 512], F32R,
                                                      tag="psa")
                                    for t in range(4):
                                        lb = 4 * lq + t
                                        nc.tensor.transpose(
                                            pt_ps[:, 128 * t:128 * (t + 1)],
                                            src[:, lb,
                                                128 * dc:128 * (dc + 1)],
                                            ident)
                                    # explicit DVE: ACT must not read f32r
                                    nc.vector.tensor_copy(
                                        dstT[:, dc,
                                             512 * lq:512 * (lq + 1)], pt_ps)

                    # ---- attention for the 8 heads of this group ----
                    with nc.named_scope(f"attn{hg}"):
                        for cbl in range(4):
                            cb = 4 * hg + cbl
                            for i in range(2):
                                h = 2 * cb + i
                                hp = 64 * i
                                nc.vector.tensor_copy(
                                    kzs[i][hp:hp + HD, :],
                                    kT[hp:hp + HD, cbl, :])
                                pso = ps_o.tile([128, L], F32, tag="pso",
                                                name=f"pso_{h}")
                                for jb in range(NJB):
                                    pt_sb = ppt.tile([128, L], BF16, tag="pt")
                                    ss = ps_s.tile([128, L], F32, tag="pss")
                                    for hf in range(2):
                                        nc.tensor.matmul(
                                            ss[:, 512 * hf:512 * (hf + 1)],
                                            lhsT=kzs[i][:,
                                                        128 * jb:128 * (jb + 1)],
                                            rhs=qT[:, cbl,
                                                   512 * hf:512 * (hf + 1)],
                                            start=True, stop=True)
                                    nc.scalar.activation(
                                        pt_sb, ss, AF.Exp,
                                        scale=fk_all[:, jb, h:h + 1])
                                    for hf in range(2):
                                        nc.tensor.matmul(
                                            pso[:, 512 * hf:512 * (hf + 1)],
                                            lhsT=vb[:, jb, h, :],
                                            rhs=pt_sb[:,
                                                      512 * hf:512 * (hf + 1)],
                                            start=(jb == 0),
                                            stop=(jb == NJB - 1))
                                srow = p1.tile([1, L], F32, tag="srow",
                                               bufs=1)
                                nc.vector.tensor_copy(srow, pso[HD:HD + 1, :])
                                rs = p1.tile([1, L], F32, tag="rs", bufs=1)
                                nc.vector.reciprocal_approx_fast(rs, srow)
                                fsb = p1.tile([HD, L], F32, tag="fsb", bufs=2)
                                nc.gpsimd.partition_broadcast(fsb, rs)
                                nc.vector.tensor_mul(
                                    outT[hp:hp + HD, cb, :],
                                    pso[0:HD, :], fsb)

                # ---------------- output projection ----------------
                with nc.named_scope("proj"):
                    for hf in range(2):
                        wpn = pw.tile([128, NCB, 512], BF16, tag="wqn",
                                      name=f"wp{hf}")
                        nc.sync.dma_start(out=wpn, in_=bass.AP(
                            tensor=wp, offset=512 * hf,
                            ap=[[C, 128], [128 * C, NCB], [1, 512]]))
                        for lb in range(NLB):
                            psy = ps_a.tile([128, 512], F32, tag="psa")
                            for cb in range(NCB):
                                nc.tensor.matmul(
                                    psy,
                                    lhsT=outT[:, cb, 128 * lb:128 * (lb + 1)],
                                    rhs=wpn[:, cb, :],
                                    start=(cb == 0), stop=(cb == NCB - 1))
                            ysb = p1.tile([128, 512], F32, tag="ysb", bufs=2)
                            nc.any.tensor_copy(ysb, psy)
                            nc.sync.dma_start(
                                out=y[128 * lb:128 * (lb + 1),
                                      512 * hf:512 * (hf + 1)],
                                in_=ysb)

    nc.compile()
    return nc


def _get_nc():
    global _nc_cache
    if _nc_cache is None:
        _nc_cache = build_nc()
    return _nc_cache


def _host_prep(x, cos, sin, w_qkv, w_proj, q_norm_w, k_norm_w):
    x = np.asarray(x, dtype=np.float32)
    cos = np.asarray(cos, dtype=np.float32)
    sin = np.asarray(sin, dtype=np.float32)
    w_qkv = np.asarray(w_qkv, dtype=np.float32)
    w_proj = np.asarray(w_proj, dtype=np.float32)
    q_norm_w = np.asarray(q_norm_w, dtype=np.float32)
    k_norm_w = np.asarray(k_norm_w, dtype=np.float32)

    bf16 = ml_dtypes.bfloat16
    wqT = np.ascontiguousarray(w_qkv.T.astype(bf16))     # (C, 3C)
    wpT = np.ascontiguousarray(w_proj.T.astype(bf16))    # (C, C)

    def fold(w):
        # cosW[l,d] = cos[l,d]*w[d]
        # sinW[l,d<32] = -sin[l,d]*w[d+32]; sinW[l,d>=32] = sin[l,d]*w[d-32]
        cosW = cos * w[None, :]
        w_rot = np.concatenate([w[32:], w[:32]])
        sinW = (sin * w_rot[None, :]).copy()
        sinW[:, :32] *= -1.0
        return (np.ascontiguousarray(cosW.astype(bf16)),
                np.ascontiguousarray(sinW.astype(bf16)))

    cqt, sqt = fold(q_norm_w)
    ckt, skt = fold(k_norm_w)

    in_maps = []
    for b in range(N_CORES):
        in_maps.append({
            "xT": np.ascontiguousarray(x[b].T.astype(bf16)),
            "wq": wqT, "wp": wpT,
            "cq": cqt, "sq": sqt, "ck": ckt, "sk": skt,
        })
    return in_maps


def kernel(x, cos, sin, w_qkv, w_proj, q_norm_w, k_norm_w, _trace=False):
    global _last_results
    nc = _get_nc()
    in_maps = _host_prep(x, cos, sin, w_qkv, w_proj, q_norm_w, k_norm_w)
    r = run_bass_kernel_spmd(nc, in_maps, list(range(N_CORES)), trace=_trace)
    _last_results = r
    return np.stack([r.results[b]["y"] for b in range(N_CORES)], axis=0)


# revision 33
# speedup vs baseline: 1.1991x; 1.1850x over previous
"""Trainium2 Bass kernel for a full attention block (QKV proj + RMSNorm + RoPE +
softmax attention + output proj), batch-data-parallel across 8 NeuronCores.

Shapes (hardcoded): x (8, 1024, 1024), H=16 heads, hd=64.
Each core processes one batch element; weights are replicated.

v2: head-group (8-head) pipelined schedule so ACT exp overlaps PE matmuls of
the other half; all matmul operands bf16 (psum stays f32); S matmuls use
K=64 tile-positioned stationaries straight out of kT (no zero-padding);
RoPE reads PSUM directly with a swapped-halves AP view (one multiply for the
rotated term); q-RMS factor folded in with a single stride-0 broadcast mul;
k-RMS folded into the exp scale (per-partition) as 1/sqrt(ss+hd*eps) which
also absorbs the 1/sqrt(hd) softmax scale; PV uses a ones-augmented V so
softmax sums land in psum row 64; transposes evacuated in batches of 4.
"""
import numpy as np
import ml_dtypes

import concourse.bass as bass
from concourse import bacc
import concourse.mybir as mybir
import concourse.tile as tile
from concourse.bass_utils import run_bass_kernel_spmd
from concourse.masks import make_identity

F32 = mybir.dt.float32
F32R = mybir.dt.float32r
BF16 = mybir.dt.bfloat16
AF = mybir.ActivationFunctionType
ALU = mybir.AluOpType

B, L, C, H, HD = 8, 1024, 1024, 16, 64
EPS = 1e-6
NLB = L // 128   # 8 l-blocks
NCB = C // 128   # 8 c-blocks
NJB = L // 128   # 8 j-blocks
N_CORES = 8

_nc_cache = None
_last_results = None  # BassKernelResults of the most recent run (for test.py)


def _bcast(ap2d, reps):
    """(128, w) AP -> (128, reps, w) stride-0 broadcast view."""
    return bass.AP(tensor=ap2d.tensor, offset=ap2d.offset,
                   ap=[ap2d.ap[0], [0, reps], ap2d.ap[1]])


def _bcast_rot(ap2d, reps):
    """(128, 64) AP -> (128, reps, 2, 32) stride-0 bcast view (hd split)."""
    return bass.AP(tensor=ap2d.tensor, offset=ap2d.offset,
                   ap=[ap2d.ap[0], [0, reps], [32, 2], [1, 32]])


def _rot_view(ps):
    """(128, 512) psum AP -> (128, 8, 2, 32) view with 32-halves swapped
    within each 64-wide head: element (p,h,r,j) = ps[p, 64h + 32(1-r) + j]."""
    return bass.AP(tensor=ps.tensor, offset=ps.offset + 32,
                   ap=[ps.ap[0], [HD, 8], [-32, 2], [1, 32]])


def _inner_bcast(ap2d, reps):
    """(128, w) AP -> (128, w, reps) stride-0 inner broadcast view."""
    return bass.AP(tensor=ap2d.tensor, offset=ap2d.offset,
                   ap=[ap2d.ap[0], ap2d.ap[1], [0, reps]])


def build_nc():
    nc = bacc.Bacc("TRN2", target_bir_lowering=False)

    xT = nc.declare_dram_parameter("xT", [C, L], BF16, isOutput=False)
    wq = nc.declare_dram_parameter("wq", [C, 3 * C], BF16, isOutput=False)
    wp = nc.declare_dram_parameter("wp", [C, C], BF16, isOutput=False)
    # RoPE tables with rms-norm weights folded in (host-prepared)
    cq = nc.declare_dram_parameter("cq", [L, HD], BF16, isOutput=False)
    sq = nc.declare_dram_parameter("sq", [L, HD], BF16, isOutput=False)
    ck = nc.declare_dram_parameter("ck", [L, HD], BF16, isOutput=False)
    sk = nc.declare_dram_parameter("sk", [L, HD], BF16, isOutput=False)
    y = nc.declare_dram_parameter("y", [L, C], F32, isOutput=True)

    def tab_view(t):
        # (L, 64) DRAM -> SBUF (128, 8, 64): element (p, lc, j) = t[128*lc + p, j]
        return bass.AP(tensor=t, offset=0,
                       ap=[[HD, 128], [128 * HD, NLB], [1, HD]])

    with tile.TileContext(nc) as tc:
        with tc.tile_pool(name="persist", bufs=1) as persist:
            # --- persistent tiles ---
            cq_sb = persist.tile([128, NLB, HD], BF16)
            sq_sb = persist.tile([128, NLB, HD], BF16)
            ck_sb = persist.tile([128, NLB, HD], BF16)
            sk_sb = persist.tile([128, NLB, HD], BF16)
            nc.sync.dma_start(out=cq_sb, in_=tab_view(cq))
            nc.sync.dma_start(out=sq_sb, in_=tab_view(sq))
            nc.sync.dma_start(out=ck_sb, in_=tab_view(ck))
            nc.sync.dma_start(out=sk_sb, in_=tab_view(sk))

            fk_all = persist.tile([128, NLB, H], F32)        # fk per (j, head)
            # V blocks padded to 128 cols (65th = ones, rest zero) so PV
            # matmuls write a full 128-partition psum
            vb = persist.tile([128, NLB, H, 128], BF16)
            nc.vector.memset(vb, 0.0)
            ident_f = persist.tile([128, 128], F32)
            make_identity(nc, ident_f)
            ident = persist.tile([128, 128], F32R)
            nc.vector.tensor_copy(ident, ident_f)
            # zero-padded K stationaries: partitions [64i, 64i+64) hold the
            # current head's K^T, the other half stays zero forever
            kz0 = persist.tile([128, L], BF16)
            nc.vector.memset(kz0, 0.0)
            kz1 = persist.tile([128, L], BF16)
            nc.vector.memset(kz1, 0.0)
            kzs = (kz0, kz1)
            eps_q = persist.tile([128, 1], F32)
            nc.vector.memset(eps_q, EPS)
            eps_k = persist.tile([128, 1], F32)
            nc.vector.memset(eps_k, HD * EPS)

            ones128 = persist.tile([128, 1], F32)
            nc.vector.memset(ones128, 1.0)
            nc.vector.tensor_copy(
                bass.AP(tensor=vb.tensor, offset=vb.offset + HD,
                        ap=[vb.ap[0], vb.ap[1], vb.ap[2], [1, 1]]),
                bass.AP(tensor=ones128.tensor, offset=ones128.offset,
                        ap=[ones128.ap[0], [0, NLB], [0, H], [1, 1]]))

            with tc.tile_pool(name="pbig", bufs=1) as pbig, \
                 tc.tile_pool(name="phalf", bufs=2) as phalf, \
                 tc.tile_pool(name="pw", bufs=2) as pw, \
                 tc.tile_pool(name="ppt", bufs=4) as ppt, \
                 tc.tile_pool(name="p1", bufs=1) as p1, \
                 tc.tile_pool(name="ps_a", bufs=2, space="PSUM") as ps_a, \
                 tc.tile_pool(name="ps_s", bufs=2, space="PSUM") as ps_s, \
                 tc.tile_pool(name="ps_o", bufs=1, space="PSUM") as ps_o:

                xr = pbig.tile([128, NCB, L], BF16, name="xr")
                outT = pbig.tile([128, NCB, L], BF16, name="outT")
                # element (p, cc, l) = xT[128*cc + p, l]; split in half so
                # the first matmuls start as soon as cb 0-3 land
                for xh in range(2):
                    nc.sync.dma_start(out=xr[:, 4 * xh:4 * (xh + 1), :],
                                      in_=bass.AP(
                        tensor=xT, offset=512 * L * xh,
                        ap=[[L, 128], [128 * L, 4], [1, L]]))

                for hg in range(2):
                    with nc.named_scope(f"qkv{hg}"):
                        qr = phalf.tile([128, NLB, 512], F32R, tag="qr",
                                        bufs=1, name=f"qr{hg}")
                        kr = phalf.tile([128, NLB, 512], F32R, tag="kr",
                                        bufs=1, name=f"kr{hg}")
                        for kind, n in (("q", hg), ("k", hg + 2), ("v", hg + 4)):
                            wqn = pw.tile([128, NCB, 512], BF16, tag="wqn",
                                          name=f"w_{kind}{hg}")
                            # element (p, cc, j) = wq[128*cc + p, 512*n + j]
                            nc.sync.dma_start(out=wqn, in_=bass.AP(
                                tensor=wq, offset=512 * n,
                                ap=[[3 * C, 128], [128 * 3 * C, NCB], [1, 512]]))
                            if kind != "v":
                                sst_all = p1.tile([128, NLB, 8], F32,
                                                  tag="sst", bufs=2)
                            for lb in range(NLB):
                                ps = ps_a.tile([128, 512], F32, tag="psa")
                                for cb in range(NCB):
                                    nc.tensor.matmul(
                                        ps,
                                        lhsT=xr[:, cb, 128 * lb:128 * (lb + 1)],
                                        rhs=wqn[:, cb, :],
                                        start=(cb == 0), stop=(cb == NCB - 1))
                                if kind == "v":
                                    nc.any.tensor_copy(
                                        vb[:, lb, 8 * hg:8 * hg + 8, 0:HD],
                                        ps.rearrange("p (h d) -> p h d", d=HD))
                                    continue
                                # stage psum out fast (frees the bank after
                                # one copy instead of three DVE reads)
                                stg = p1.tile([128, 512], BF16, tag="stage",
                                              bufs=3)
                                nc.any.tensor_copy(stg, ps)
                                # rms stats (Square shares the exp ACT table
                                # set; Sqrt is deferred + batched per slice)
                                sqt = p1.tile([128, 512], BF16, tag="sqt",
                                              bufs=2)
                                nc.scalar.activation(sqt, stg, AF.Square)
                                nc.vector.tensor_reduce(
                                    sst_all[:, lb, :],
                                    sqt.rearrange("p (h d) -> p h d", d=HD),
                                    axis=mybir.AxisListType.X, op=ALU.add)
                                s3 = stg.rearrange("p (h d) -> p h d", d=HD)
                                cos_sb = cq_sb if kind == "q" else ck_sb
                                sin_sb = sq_sb if kind == "q" else sk_sb
                                dst = qr if kind == "q" else kr
                                a_t = p1.tile([128, 8, HD], BF16,
                                              tag="ropeA", bufs=2)
                                nc.vector.tensor_mul(
                                    a_t, s3, _bcast(cos_sb[:, lb, :], 8))
                                b_t = p1.tile([128, 8, 2, 32], BF16,
                                              tag="ropeB", bufs=2)
                                nc.vector.tensor_mul(
                                    b_t, _rot_view(stg),
                                    _bcast_rot(sin_sb[:, lb, :], 8))
                                nc.vector.tensor_add(
                                    dst[:, lb, :].rearrange(
                                        "p (h d) -> p h d", d=HD),
                                    a_t,
                                    b_t.rearrange("p h r j -> p h (r j)"))
                            # batched slice-wide rsqrt (one Sqrt per slice
                            # keeps ACT table switches rare)
                            if kind == "q":
                                rt = p1.tile([128, NLB, 8], F32, tag="rt",
                                             bufs=2)
                                nc.scalar.activation(
                                    rt.rearrange("p l h -> p (l h)"),
                                    sst_all.rearrange("p l h -> p (l h)"),
                                    AF.Sqrt, scale=1.0 / HD, bias=eps_q)
                                fqv = p1.tile([128, NLB, 8], F32, tag="fqv",
                                              bufs=2)
                                nc.vector.reciprocal_approx_fast(
                                    fqv.rearrange("p l h -> p (l h)"),
                                    rt.rearrange("p l h -> p (l h)"))
                                for lb in range(NLB):
                                    # fold fq into q in place
                                    nc.vector.tensor_mul(
                                        qr[:, lb, :].rearrange(
                                            "p (h d) -> p h d", d=HD),
                                        qr[:, lb, :].rearrange(
                                            "p (h d) -> p h d", d=HD),
                                        _inner_bcast(fqv[:, lb, :], HD))
                            elif kind == "k":
                                # fk = 1/sqrt(ss + hd*eps)  (= rms_k/8;
                                # absorbs the 1/sqrt(hd) softmax scale)
                                rt = p1.tile([128, NLB, 8], F32, tag="rt",
                                             bufs=2)
                                nc.scalar.activation(
                                    rt.rearrange("p l h -> p (l h)"),
                                    sst_all.rearrange("p l h -> p (l h)"),
                                    AF.Sqrt, scale=1.0, bias=eps_k)
                                nc.vector.reciprocal_approx_fast(
                                    fk_all[:, :, 8 * hg:8 * hg + 8],
                                    rt)

                    # ---- transposes for this head group ----
                    qT = phalf.tile([128, 4, L], BF16, tag="qT", name=f"qT{hg}")
                    kT = phalf.tile([128, 4, L], BF16, tag="kT", name=f"kT{hg}")
                    with nc.named_scope(f"tr{hg}"):
                        for src, dstT in ((qr, qT), (kr, kT)):
                            for dc in range(4):
                                for lq in range(2):
                                    pt_ps = ps_a.tile([128, 512], F32R,
                                                      tag="psa")
                                    for t in range(4):
                                        lb = 4 * lq + t
                                        nc.tensor.transpose(
                                            pt_ps[:, 128 * t:128 * (t + 1)],
                                            src[:, lb,
                                                128 * dc:128 * (dc + 1)],
                                            ident)
                                    # explicit DVE: ACT must not read f32r
                                    nc.vector.tensor_copy(
                                        dstT[:, dc,
                                             512 * lq:512 * (lq + 1)], pt_ps)

                    # ---- attention for the 8 heads of this group ----
                    with nc.named_scope(f"attn{hg}"):
                        for cbl in range(4):
                            cb = 4 * hg + cbl
                            for i in range(2):
                                h = 2 * cb + i
                                hp = 64 * i
                                nc.vector.tensor_copy(
                                    kzs[i][hp:hp + HD, :],
                                    kT[hp:hp + HD, cbl, :])
                                pso = ps_o.tile([128, L], F32, tag="pso",
                                                name=f"pso_{h}")
                                for jb in range(NJB):
                                    pt_sb = ppt.tile([128, L], BF16, tag="pt")
                                    ss = ps_s.tile([128, L], F32, tag="pss")
                                    for hf in range(2):
                                        nc.tensor.matmul(
                                            ss[:, 512 * hf:512 * (hf + 1)],
                                            lhsT=kzs[i][:,
                                                        128 * jb:128 * (jb + 1)],
                                            rhs=qT[:, cbl,
                                                   512 * hf:512 * (hf + 1)],
                                            start=True, stop=True)
                                    nc.scalar.activation(
                                        pt_sb, ss, AF.Exp,
                                        scale=fk_all[:, jb, h:h + 1])
                                    for hf in range(2):
                                        nc.tensor.matmul(
                                            pso[:, 512 * hf:512 * (hf + 1)],
                                            lhsT=vb[:, jb, h, :],
                                            rhs=pt_sb[:,
                                                      512 * hf:512 * (hf + 1)],
                                            start=(jb == 0),
                                            stop=(jb == NJB - 1))
                                srow = p1.tile([1, L], F32, tag="srow",
                                               bufs=1)
                                nc.vector.tensor_copy(srow, pso[HD:HD + 1, :])
                                rs = p1.tile([1, L], F32, tag="rs", bufs=1)
                                nc.vector.reciprocal_approx_fast(rs, srow)
                                fsb = p1.tile([HD, L], F32, tag="fsb", bufs=2)
                                nc.gpsimd.partition_broadcast(fsb, rs)
                                nc.vector.tensor_mul(
                                    outT[hp:hp + HD, cb, :],
                                    pso[0:HD, :], fsb)

                # ---------------- output projection ----------------
                with nc.named_scope("proj"):
                    wpns = []
                    for hf in range(2):
                        wpn = pw.tile([128, NCB, 512], BF16, tag="wqn",
                                      name=f"wp{hf}")
                        nc.sync.dma_start(out=wpn, in_=bass.AP(
                            tensor=wp, offset=512 * hf,
                            ap=[[C, 128], [128 * C, NCB], [1, 512]]))
                        wpns.append(wpn)
                    # partial proj over cb 0-3 (head group 0): depends only
                    # on the first 8 heads, so these matmuls fill PE gaps
                    # during head-group-1 attention and keep HAM warm.
                    # Reuses xr's SBUF slot (xr is dead after qkv1).
                    ypart = pbig.tile([128, 2, NLB, 512], BF16, tag="xr",
                                      name="ypart")
                    for hf in range(2):
                        for lb in range(NLB):
                            psy = ps_a.tile([128, 512], F32, tag="psa")
                            for cb in range(4):
                                nc.tensor.matmul(
                                    psy,
                                    lhsT=outT[:, cb, 128 * lb:128 * (lb + 1)],
                                    rhs=wpns[hf][:, cb, :],
                                    start=(cb == 0), stop=(cb == 3))
                            nc.any.tensor_copy(ypart[:, hf, lb, :], psy)
                    for hf in range(2):
                        for lb in range(NLB):
                            psy = ps_a.tile([128, 512], F32, tag="psa")
                            for cb in range(4, NCB):
                                nc.tensor.matmul(
                                    psy,
                                    lhsT=outT[:, cb, 128 * lb:128 * (lb + 1)],
                                    rhs=wpns[hf][:, cb, :],
                                    start=(cb == 4), stop=(cb == NCB - 1))
                            ysb = p1.tile([128, 512], F32, tag="ysb", bufs=2)
                            nc.vector.tensor_add(ysb, psy,
                                                 ypart[:, hf, lb, :])
                            nc.sync.dma_start(
                                out=y[128 * lb:128 * (lb + 1),
                                      512 * hf:512 * (hf + 1)],
                                in_=ysb)

    nc.compile()
    return nc


def _get_nc():
    global _nc_cache
    if _nc_cache is None:
        _nc_cache = build_nc()
    return _nc_cache


def _host_prep(x, cos, sin, w_qkv, w_proj, q_norm_w, k_norm_w):
    x = np.asarray(x, dtype=np.float32)
    cos = np.asarray(cos, dtype=np.float32)
    sin = np.asarray(sin, dtype=np.float32)
    w_qkv = np.asarray(w_qkv, dtype=np.float32)
    w_proj = np.asarray(w_proj, dtype=np.float32)
    q_norm_w = np.asarray(q_norm_w, dtype=np.float32)
    k_norm_w = np.asarray(k_norm_w, dtype=np.float32)

    bf16 = ml_dtypes.bfloat16
    wqT = np.ascontiguousarray(w_qkv.T.astype(bf16))     # (C, 3C)
    wpT = np.ascontiguousarray(w_proj.T.astype(bf16))    # (C, C)

    def fold(w):
        # cosW[l,d] = cos[l,d]*w[d]
        # sinW[l,d<32] = -sin[l,d]*w[d+32]; sinW[l,d>=32] = sin[l,d]*w[d-32]
        cosW = cos * w[None, :]
        w_rot = np.concatenate([w[32:], w[:32]])
        sinW = (sin * w_rot[None, :]).copy()
        sinW[:, :32] *= -1.0
        return (np.ascontiguousarray(cosW.astype(bf16)),
                np.ascontiguousarray(sinW.astype(bf16)))

    cqt, sqt = fold(q_norm_w)
    ckt, skt = fold(k_norm_w)

    in_maps = []
    for b in range(N_CORES):
        in_maps.append({
            "xT": np.ascontiguousarray(x[b].T.astype(bf16)),
            "wq": wqT, "wp": wpT,
            "cq": cqt, "sq": sqt, "ck": ckt, "sk": skt,
        })
    return in_maps


def kernel(x, cos, sin, w_qkv, w_proj, q_norm_w, k_norm_w, _trace=False):
    global _last_results
    nc = _get_nc()
    in_maps = _host_prep(x, cos, sin, w_qkv, w_proj, q_norm_w, k_norm_w)
    r = run_bass_kernel_spmd(nc, in_maps, list(range(N_CORES)), trace=_trace)
    _last_results = r
    return np.stack([r.results[b]["y"] for b in range(N_CORES)], axis=0)
